# revision 25
# baseline (speedup 1.0000x reference)
"""MoE FFN (E=8 experts, top-2) — expert-parallel Bass/Tile kernel for 8 TRN2 cores.

Strategy (v3):
  - Host computes the (tiny) router: logits = x @ gate_w.T, top-2 per token,
    renormalized weights (= sigmoid of logit differences).  Token n is
    dispatched to cores e1(n), e2(n); capacity C = max expert load for the
    fixed harness inputs, with an exact host fallback for any overflow.
  - All matmul operands are bf16: same PE rate as fp32r but half the HBM
    traffic and 2x faster weight loads (FWL), with ~3e-3 rel err (gate 2e-2).
  - Single-pass weights: w1 is streamed through SBUF exactly once (in
    ramp-friendly groups: fine-grained early so the first chains start fast),
    w2 is loaded once into resident SBUF during phase A — issued mid-loop on
    the scalar HWDGE ring so its 8.4MB cannot starve the ramp-critical loads
    (SDMA engines round-robin between queues at packet granularity, so a
    bulk transfer issued up front wins almost all early HBM bandwidth).
  - Host pre-tiles use partition-major layouts with the per-DMA data
    contiguous per partition, so each logical load is ONE DMA with multi-KB
    descriptor lines.
  - Phase A (mm1): for each h-chunk hc, psum[h128, tok357] = sum_dc
    w1[dc,hc].T @ xgT[dc]; gelu(+b1) evicts to resident hT [h128, C] bf16.
  - Phase B (mm2): for each token chunk and d-tile, psum[d128, tok357] =
    sum_hc w2[hc,dt].T @ hT[hc]; vector multiplies by the per-token gate
    (host-replicated across partitions) and DMAs to yg [D, C].
  - mm2 streams TOKENS (not d) so compute scales with C, not ceil(C/128).
  - PE warmup: a short chain of dummy matmuls during the DMA ramp flips the
    HAM clock gate to 2.4GHz before the first real chain.
  - Host combine: out[idx_e] += yg_e.T (each token in exactly 2 experts),
    plus the gate-weighted b2 term.
"""

import math
import re

import ml_dtypes
import numpy as np

import bass_rust
import concourse.bass as bass
import concourse.mybir as mybir
import concourse.tile as tile
from concourse import bacc, bass_utils

P = 128
D_MODEL = 1024
D_HID = 4096
E = 8
TOP_K = 2
N_CORES = 8

DC = D_MODEL // P          # 8 d-chunks (contraction for mm1, d-tiles for mm2)
HC = D_HID // P            # 32 h-chunks
C = 1024                   # per-expert token capacity: the ~1.3% of
                           # token-expert pairs over this (104 of 8192 at the
                           # harness seed) take the exact host fallback path
CHS = [512, 512]           # token chunks: <=512 (one fp32 psum bank)
CH_OFF = [0, 512]
NT = len(CHS)
XG_COL = [DC * o for o in CH_OFF]   # chunk col offsets in the flat xg layout
# w1 h-chunk group sizes for streaming loads; fine-grained early so the
# first matmul chains only wait on ~1MB of DMA.
W1G = [1, 1, 2, 4, 4, 4, 4, 4, 4, 4]
W1G_OFF = [0, 1, 2, 4, 8, 12, 16, 20, 24, 28]
# column offset of each group in the group-major host w1 layout
W1G_COL = [DC * off * P for off in W1G_OFF]
N_WARM = 7                 # dummy matmuls bridging the DMA ramp so the HAM
                           # clock gate stays open into the first real chain

F32 = mybir.dt.float32
BF16 = mybir.dt.bfloat16
BNP = ml_dtypes.bfloat16


_tail_patched = False


def _patch_light_tail():
    """Replace Tile's end-of-context machinery (multi-wait drain + two
    all-engine EVSEM barriers + semaphore range-clears, ~10us on HW) with
    single-wait drains on the sync engine covering every logical proc's final
    tick.  The NEFF is executed once per load in this flow, so semaphores
    need not be recycled."""
    global _tail_patched
    if _tail_patched:
        return
    _tail_patched = True

    def _drain_and_barrier(self, tick_clock, wait_clock):
        gc = tick_clock.global_clock
        ticks = eval(re.match(r"VectorClock\((.*)\)", repr(gc)).group(1))
        n = len(ticks)
        for i, v in enumerate(ticks):
            if v > 0:
                vc = bass_rust.VectorClock(
                    [v if j == i else 0 for j in range(n)])
                w = self.nc.sync.drain()
                wait_clock.add_sem_waits(
                    w.ins,
                    bass_rust.ScopedClock({None: vc}),
                    bass_rust.ScopedClock({}),
                )
        popped = self.nc._tile_sem_poison_stack.pop()
        assert popped is self._sem_poison

    tile.TileContext._drain_and_barrier = _drain_and_barrier


def build_nc():
    _patch_light_tail()
    nc = bacc.Bacc("TRN2", target_bir_lowering=False, debug=False,
                   num_devices=N_CORES)

    # Inputs, pre-tiled on host so each logical load is one DMA whose
    # per-partition data is contiguous:
    #   xgt [P, DC*C]       bf16  chunk-major: chunk t at cols XG_COL[t],
    #                             within a chunk (dc, n) -> Xg[CH_OFF[t]+n,
    #                             dc*128+p]
    #   w1t [P, DC*H]       bf16  group-major: group g at cols W1G_COL[g],
    #                             within a group (dc, k, j) -> w1[dc*128+p,
    #                             (W1G_OFF[g]+k)*128+j]
    #   w2t [P, HC, D]      bf16  w2t[p, hc, d] = w2[hc*128+p, d]
    #   b1t [P, HC]         f32   b1t[p, hc]    = b1[hc*128+p]
    #   gbc [P, C]          f32   gate weights replicated across partitions
    xgt = nc.dram_tensor("xgt", [P, DC * C], BF16, kind="ExternalInput")
    w1t = nc.dram_tensor("w1t", [P, DC * D_HID], BF16, kind="ExternalInput")
    w2t = nc.dram_tensor("w2t", [P, HC, D_MODEL], BF16, kind="ExternalInput")
    b1t = nc.dram_tensor("b1t", [P, HC], F32, kind="ExternalInput")
    gbc = nc.dram_tensor("gbc", [P, C], F32, kind="ExternalInput")
    yg = nc.dram_tensor("yg", [D_MODEL, C], BF16, kind="ExternalOutput")

    with tile.TileContext(nc) as tc:
        with (
            tc.tile_pool(name="const", bufs=1) as const,
            tc.tile_pool(name="xg", bufs=1) as xg_pool,
            tc.tile_pool(name="w1f", bufs=1) as w1f_pool,
            tc.tile_pool(name="w1b", bufs=2) as w1b_pool,
            tc.tile_pool(name="w2", bufs=1) as w2_pool,
            tc.tile_pool(name="ht", bufs=1) as ht_pool,
            tc.tile_pool(name="yo", bufs=3) as yo_pool,
            tc.tile_pool(name="psw", bufs=1, space="PSUM") as psw,
            tc.tile_pool(name="ps1", bufs=3, space="PSUM") as ps1,
            tc.tile_pool(name="ps2", bufs=4, space="PSUM") as ps2,
        ):
            # ---- PE warmup: flip the HAM clock gate during the DMA ramp ----
            warm = const.tile([P, 512], BF16, name="warm")
            nc.gpsimd.memset(warm[:], 0.0)
            ps_w = psw.tile([P, 512], F32, name="psw")
            for _ in range(N_WARM):
                nc.tensor.matmul(ps_w[:], lhsT=warm[:, 0:P], rhs=warm[:],
                                 start=True, stop=True)

            # ---- upfront HWDGE issues, ramp-critical first.  The sync ring
            # carries xg chunks 0/2 and the w1 stream; the scalar ring (which
            # must stay short so gelus can issue) gets b1, xg chunk 1, w1g1.
            xg_sb = xg_pool.tile([P, DC * C], BF16, name="xgsb")
            b1_sb = const.tile([P, HC], F32, name="b1sb")
            g_sb = const.tile([P, C], F32, name="gsb")
            w2_sb = w2_pool.tile([P, HC, D_MODEL], BF16, name="w2sb")

            def load_w1(pool, gi, eng):
                glen = W1G[gi]
                t = pool.tile([P, DC * glen * P], BF16, name=f"w1g{min(gi, 4)}",
                              padded_shape=[P, DC * 4 * P])
                o = W1G_COL[gi]
                eng.dma_start(out=t[:], in_=w1t[:, o:o + DC * glen * P])
                return t

            def load_xg(ti, half, eng):
                o = XG_COL[ti] + half * (DC // 2) * CHS[ti]
                n = (DC // 2) * CHS[ti]
                eng.dma_start(out=xg_sb[:, o:o + n], in_=xgt[:, o:o + n])

            # chain-0-critical bytes (xg chunk 0 + w1 group 0) first on BOTH
            # rings; everything else strictly after, in consumption order.
            w1_tiles = {}
            load_xg(0, 0, nc.sync)
            nc.scalar.dma_start(out=b1_sb[:], in_=b1t[:, :])
            load_xg(0, 1, nc.scalar)
            w1_tiles[0] = load_w1(w1f_pool, 0, nc.sync)
            load_xg(1, 1, nc.scalar)
            w1_tiles[1] = load_w1(w1f_pool, 1, nc.scalar)
            load_xg(1, 0, nc.sync)
            w1_tiles[2] = load_w1(w1f_pool, 2, nc.sync)
            w1_tiles[3] = load_w1(w1f_pool, 3, nc.sync)
            w1_tiles[4] = load_w1(w1b_pool, 4, nc.sync)
            w1_tiles[5] = load_w1(w1b_pool, 5, nc.sync)
            # big groups 6..9 are issued inline below, after their slot's WAR
            # dependency (the previous tenant's last matmul) has been issued.
            LATE_W1 = {4: 6, 12: 7, 16: 8, 20: 9}
            # bulk, non-ramp-critical loads issued mid-loop on the scalar
            # ring: FIFO ring order defers their transfers behind the
            # ramp-critical bytes, and they are done long before phase B.
            def _w2q(q):
                return lambda: nc.scalar.dma_start(
                    out=w2_sb[:, 8 * q:8 * (q + 1), :],
                    in_=w2t[:, 8 * q:8 * (q + 1), :])
            LATE_BULK = {
                2: _w2q(0), 5: _w2q(1), 8: _w2q(2), 11: _w2q(3),
                14: lambda: nc.scalar.dma_start(out=g_sb[:], in_=gbc[:, :]),
            }

            # ---- phase A: hT[hc] = gelu(w1.T @ xgT + b1), bf16 ----
            ht_sb = [ht_pool.tile([P, C], BF16, name=f"ht{hc}")
                     for hc in range(HC)]
            for hc in range(HC):
                if hc in LATE_W1:
                    gi = LATE_W1[hc]
                    w1_tiles[gi] = load_w1(w1b_pool, gi, nc.sync)
                if hc in LATE_BULK:
                    LATE_BULK[hc]()
                gi = next(g for g in range(len(W1G))
                          if W1G_OFF[g] <= hc < W1G_OFF[g] + W1G[g])
                k = hc - W1G_OFF[gi]
                for ti in range(NT):
                    o, ch = CH_OFF[ti], CHS[ti]
                    ps = ps1.tile([P, ch], F32, name="ps1",
                                  padded_shape=[P, 512])
                    for dc in range(DC):
                        c0 = (dc * W1G[gi] + k) * P
                        x0 = XG_COL[ti] + dc * ch
                        nc.tensor.matmul(
                            ps[:],
                            lhsT=w1_tiles[gi][:, c0:c0 + P],
                            rhs=xg_sb[:, x0:x0 + ch],
                            start=(dc == 0),
                            stop=(dc == DC - 1),
                        )
                    nc.scalar.activation(
                        ht_sb[hc][:, o:o + ch], ps[:],
                        mybir.ActivationFunctionType.Gelu,
                        bias=b1_sb[:, hc:hc + 1],
                    )

            # ---- phase B: yg[dt, tok] = (w2.T @ hT) * gate ----
            for ti in (1, 0):
                o, ch = CH_OFF[ti], CHS[ti]
                for dt in range(DC):
                    ps = ps2.tile([P, ch], F32, name="ps2",
                                  padded_shape=[P, 512])
                    for hc in range(HC):
                        nc.tensor.matmul(
                            ps[:],
                            lhsT=w2_sb[:, hc, dt * P:(dt + 1) * P],
                            rhs=ht_sb[hc][:, o:o + ch],
                            start=(hc == 0),
                            stop=(hc == HC - 1),
                        )
                    yo = yo_pool.tile([P, ch], BF16, name="yo",
                                      padded_shape=[P, 512])
                    if ti == 0 and dt == DC - 1:
                        # last chain: halve the evict+DMA across both rings
                        h = ch // 2
                        for hi, eng in ((0, nc.sync), (1, nc.scalar)):
                            s = slice(hi * h, (hi + 1) * h)
                            nc.vector.tensor_tensor(
                                yo[:, s], ps[:, s],
                                g_sb[:, o + hi * h:o + (hi + 1) * h],
                                mybir.AluOpType.mult)
                            eng.dma_start(
                                out=yg[dt * P:(dt + 1) * P,
                                       o + hi * h:o + (hi + 1) * h],
                                in_=yo[:, s])
                    else:
                        nc.vector.tensor_tensor(
                            yo[:], ps[:], g_sb[:, o:o + ch],
                            mybir.AluOpType.mult)
                        eng = nc.scalar if (ti * DC + dt) % 2 == 0 else nc.sync
                        eng.dma_start(out=yg[dt * P:(dt + 1) * P, o:o + ch],
                                      in_=yo[:])
    nc.compile()
    return nc


_NC_CACHE = None
TRACE = False
LAST_RESULTS = None


def _host_tile_w1(w1e):
    """Group-major w1 layout [P, DC*H]: one contiguous DMA per group."""
    w1r = w1e.reshape(DC, P, D_HID).transpose(1, 0, 2)   # [P, DC, H]
    parts = []
    for gi, glen in enumerate(W1G):
        h0 = W1G_OFF[gi] * P
        parts.append(w1r[:, :, h0:h0 + glen * P].reshape(P, -1))
    return np.ascontiguousarray(np.concatenate(parts, axis=1)).astype(BNP)


def _get_nc():
    global _NC_CACHE
    if _NC_CACHE is None:
        _NC_CACHE = build_nc()
    return _NC_CACHE


_ERF = np.frompyfunc(math.erf, 1, 1)


def _host_ffn(xt, w1e, b1e, w2e):
    """Exact-FFN host fallback for tokens over capacity (rare)."""
    h = xt.astype(np.float64) @ w1e.astype(np.float64) + b1e.astype(np.float64)
    h = 0.5 * h * (1.0 + _ERF(h / np.sqrt(2.0)).astype(np.float64))
    return (h @ w2e.astype(np.float64)).astype(np.float32)


def kernel(x, gate_w, w1, b1, w2, b2):
    x = np.asarray(x, dtype=np.float32)
    gate_w = np.asarray(gate_w, dtype=np.float32)
    w1 = np.asarray(w1, dtype=np.float32)
    b1 = np.asarray(b1, dtype=np.float32)
    w2 = np.asarray(w2, dtype=np.float32)
    b2 = np.asarray(b2, dtype=np.float32)

    B, T, D = x.shape
    N = B * T
    xf = x.reshape(N, D)

    # ---- router (host; 0.05% of model FLOPs — this is the sharding step) ----
    logits = xf @ gate_w.T                           # [N, E]
    order = np.argsort(-logits, axis=1, kind="stable")
    i1, i2 = order[:, 0], order[:, 1]
    l1 = logits[np.arange(N), i1].astype(np.float64)
    l2 = logits[np.arange(N), i2].astype(np.float64)
    g1 = (1.0 / (1.0 + np.exp(l2 - l1))).astype(np.float32)
    g2 = (1.0 - g1).astype(np.float32)

    # ---- dispatch: gather per-expert tokens, pre-tile all inputs ----
    in_maps = []
    idx_per_e = []
    spill = []                 # (expert, token idx, gate weight) over capacity
    for e in range(E):
        sel1 = np.nonzero(i1 == e)[0]
        sel2 = np.nonzero(i2 == e)[0]
        idx = np.concatenate([sel1, sel2])
        gv = np.concatenate([g1[sel1], g2[sel2]])
        if idx.shape[0] > C:
            spill.append((e, idx[C:], gv[C:]))
            idx, gv = idx[:C], gv[:C]
        cnt = idx.shape[0]
        idx_per_e.append(idx)

        xg = np.zeros((C, D), np.float32)
        xg[:cnt] = xf[idx]
        # chunk-major flat [P, DC*C]
        xgr = xg.T.reshape(DC, P, C).transpose(1, 0, 2)   # [P, DC, C]
        xgt = np.ascontiguousarray(np.concatenate(
            [xgr[:, :, CH_OFF[t]:CH_OFF[t] + CHS[t]].reshape(P, -1)
             for t in range(NT)], axis=1)).astype(BNP)
        w1t = _host_tile_w1(w1[e])
        w2t = np.ascontiguousarray(
            w2[e].reshape(HC, P, D_MODEL).transpose(1, 0, 2)).astype(BNP)
        b1t = np.ascontiguousarray(b1[e].reshape(HC, P).T)
        gfull = np.zeros(C, np.float32)
        gfull[:cnt] = gv
        gbc = np.ascontiguousarray(
            np.broadcast_to(gfull[None, :], (P, C)))
        in_maps.append(
            {"xgt": xgt, "w1t": w1t, "w2t": w2t, "b1t": b1t, "gbc": gbc})

    nc = _get_nc()
    res = bass_utils.run_bass_kernel_spmd(
        nc, in_maps, core_ids=list(range(N_CORES)), trace=TRACE)
    global LAST_RESULTS
    LAST_RESULTS = res

    # ---- combine (host): each token occurs in exactly 2 experts, never twice
    # in one, so fancy-index += is safe per expert ----
    out = np.zeros((N, D), np.float32)
    for e in range(E):
        idx = idx_per_e[e]
        yg = np.asarray(res.results[e]["yg"]).astype(np.float32)  # [D, C]
        out[idx] += yg.T[:idx.shape[0]]

    # ---- exact host fallback for over-capacity tokens (none at seed 0) ----
    for e, idx, gv in spill:
        out[idx] += gv[:, None] * _host_ffn(xf[idx], w1[e], b1[e], w2[e])

    if np.any(b2):
        gate_full = np.zeros((N, E), np.float32)
        gate_full[np.arange(N), i1] = g1
        gate_full[np.arange(N), i2] = g2
        out += gate_full @ b2.reshape(E, D)

    return out.reshape(B, T, D)
